# revision 52
# baseline (speedup 1.0000x reference)
"""Trainium2 Bass kernel for Llama-style GQA attention (B=1, S=2048, D=4096,
32 Q heads / 8 KV heads, head_dim 128, RoPE, additive mask, causal-aware).

Sharding: 8-way tensor-parallel over heads. Core c computes Q heads 4c..4c+3
and KV head c end-to-end (projections + RoPE + attention + its rows of wo),
producing a partial [S, D] output; the host sums the 8 partials (the
all-reduce of the row-parallel wo).

v3 strategy (bf16 matmul operands, fp32 PSUM accumulation; the PE streams
1 element/cycle at 2.4GHz regardless of dtype, so the row count is the
floor — bf16 buys hidden FWL weight loads and half the DMA bytes):
  - Pass 0: K/V projections + RoPE-k + V DMA-xbar-transposes for the whole
    sequence (2 PSUM banks). Pass 1: per group G, Q-projections of G+1 are
    emitted before attention of G, so RoPE (DVE) never gates the PE and the
    8 PSUM banks are never over-subscribed (4 for Q-proj + ~4 for attention).
    xT is streamed from DRAM twice (DMA has 2x headroom; PSUM does not).
  - RoPE's even/odd interleave is folded into a column permutation of wq/wk;
    1/sqrt(head_dim) folded into wq on host.
  - Scores transposed: ST[sk, sq] = K @ Q^T; softmax sum over sk is a
    ones-column matmul; probabilities feed PV directly (ctxT = V^T @ expST).
  - Softmax division: reciprocal_approx_fast on DVE (single custom op),
    partition-broadcast via rank-1 ones matmul, deferred one head so the PE
    never waits on it.
  - Masked key-tiles skipped; diagonal tiles use multiplicative exp(mask)
    patterns on the DVE, ordered first in each head's m-loop to hide latency.
  - Pass C: wo resident in SBUF; m-outer loop collects a full [128, 4096]
    bf16 row-block and writes it with a single descriptor-friendly DMA.
"""

import math
import os
import numpy as np

# A wedged/reset device can come back one pstate down (PE at 2.0 instead of
# 2.4 GHz); ask the runtime for a clean core state before NRT init.
os.environ.setdefault("NEURON_RT_RESET_CORES", "1")

P = 128          # SBUF partitions / head_dim / tile edge
S = 2048         # sequence length
D = 4096         # model dim
HD = 128         # head dim
N_HEADS = 32
N_KV = 8
N_CORES = 8
NH_LOC = N_HEADS // N_CORES   # 4 local Q heads
SG = 512         # score/free-dim group width (one PSUM bank of fp32)
NG = S // SG     # 4 q-position groups
KT = D // P      # 32 contraction tiles for projections
NSK = S // P     # 16 key tiles

_CACHE = {}


def _classify_mask(mask):
    """Classify each [P, SG] block of mask.T into skip / plain / masked.

    Returns (sk_lists, patterns):
      sk_lists[G] = list of (m, pat_idx_or_None) key-tiles to compute for
                    query group G, and patterns = [P, SG] multiplicative
                    exp(mask) blocks (deduped).
    """
    mt = np.ascontiguousarray(mask.T.astype(np.float32))
    patterns = []
    pat_idx = {}
    sk_lists = []
    for G in range(NG):
        lst = []
        for m in range(NSK):
            blk = mt[m * P:(m + 1) * P, G * SG:(G + 1) * SG]
            if np.all(np.isneginf(blk)):
                continue
            if np.all(blk == 0.0):
                lst.append((m, None))
                continue
            with np.errstate(over="ignore"):
                pat = np.exp(blk).astype(np.float32)
            key = pat.tobytes()
            if key not in pat_idx:
                pat_idx[key] = len(patterns)
                patterns.append(pat)
            lst.append((m, pat_idx[key]))
        sk_lists.append(lst)
    return sk_lists, patterns


def _build_program(sk_lists, n_pat):
    import concourse.tile as tile
    from concourse import bacc, mybir
    from contextlib import ExitStack

    f32 = mybir.dt.float32
    bf = mybir.dt.bfloat16
    Exp = mybir.ActivationFunctionType.Exp

    nc = bacc.Bacc()
    xt_d = nc.dram_tensor("xt", [P, NG * KT * SG], bf, kind="ExternalInput")
    wq_d = nc.dram_tensor("wq", [P, KT * NH_LOC * HD], bf, kind="ExternalInput")
    wk_d = nc.dram_tensor("wk", [P, KT * HD], bf, kind="ExternalInput")
    wv_d = nc.dram_tensor("wv", [P, KT * HD], bf, kind="ExternalInput")
    wo_d = nc.dram_tensor("wo", [P, (D // SG) * NH_LOC * SG], bf,
                          kind="ExternalInput")
    cs_d = nc.dram_tensor("cs", [P, S], f32, kind="ExternalInput")
    mb_d = None
    if n_pat:
        mb_d = nc.dram_tensor("mb", [n_pat, P, SG], bf, kind="ExternalInput")
    out_d = nc.dram_tensor("out", [S, D], bf, kind="ExternalOutput")

    XCH = 4 * SG     # xT DMA chunk: 4 k-tiles, 4KB per partition line

    with ExitStack() as ctx:
        tc = ctx.enter_context(tile.TileContext(nc))
        consts = ctx.enter_context(tc.tile_pool(name="consts", bufs=1))
        kv = ctx.enter_context(tc.tile_pool(name="kv", bufs=1))
        xp = ctx.enter_context(tc.tile_pool(name="xp", bufs=10))
        qp = ctx.enter_context(tc.tile_pool(name="qp", bufs=10))
        rp = ctx.enter_context(tc.tile_pool(name="rp", bufs=4))
        ep = ctx.enter_context(tc.tile_pool(name="ep", bufs=4))
        sp = ctx.enter_context(tc.tile_pool(name="sp", bufs=4))
        cp = ctx.enter_context(tc.tile_pool(name="cp", bufs=4))
        ps = ctx.enter_context(tc.tile_pool(name="ps", bufs=8, space="PSUM"))

        # resident weights / constants on the scalar (ACT) DMA ring, loaded
        # once up front; the xT stream and all mid-kernel DMA own the sync
        # ring so the ACT engine never dispatches descriptors mid-kernel.
        wk_sb = consts.tile([P, KT * HD], bf)
        wv_sb = consts.tile([P, KT * HD], bf)
        # tiny lead pieces so the first matmuls start as early as possible
        nc.scalar.dma_start(wk_sb[:, 0:2 * HD], wk_d[:, 0:2 * HD])
        nc.scalar.dma_start(wv_sb[:, 0:2 * HD], wv_d[:, 0:2 * HD])
        half = KT * HD // 2
        nc.scalar.dma_start(wk_sb[:, 2 * HD:half], wk_d[:, 2 * HD:half])
        nc.scalar.dma_start(wv_sb[:, 2 * HD:half], wv_d[:, 2 * HD:half])
        for i in (1,):
            nc.scalar.dma_start(wk_sb[:, i * half:(i + 1) * half],
                                wk_d[:, i * half:(i + 1) * half])
            nc.scalar.dma_start(wv_sb[:, i * half:(i + 1) * half],
                                wv_d[:, i * half:(i + 1) * half])
        cs_sb = consts.tile([P, S], f32)
        nc.scalar.dma_start(cs_sb[:], cs_d[:, :])
        # wq / mb / wo loads are dispatched mid-kernel, staged to keep HBM
        # reads out of the bandwidth-critical start window (the first two
        # kv groups consume xT at nearly full HBM rate)
        wq_sb = consts.tile([P, KT * NH_LOC * HD], bf)
        qqt = KT * NH_LOC * HD // 8
        mb_sb = None
        if n_pat:
            mb_sb = consts.tile([P, n_pat * SG], bf, name="mb_sb")
        wo_sb = consts.tile([P, (D // SG) * NH_LOC * SG], bf)

        def load_wq(half):
            for i in range(4 * half, 4 * half + 4):
                nc.scalar.dma_start(wq_sb[:, i * qqt:(i + 1) * qqt],
                                    wq_d[:, i * qqt:(i + 1) * qqt])

        def load_mb():
            for i in range(n_pat):
                nc.scalar.dma_start(mb_sb[:, i * SG:(i + 1) * SG], mb_d[i])

        def load_wo():
            for i in range(8):
                nc.scalar.dma_start(wo_sb[:, i * qqt:(i + 1) * qqt],
                                    wo_d[:, i * qqt:(i + 1) * qqt])
        ones_col = consts.tile([P, 1], bf)
        nc.vector.memset(ones_col[:], 1.0)
        ones_row = consts.tile([1, P], bf)
        nc.vector.memset(ones_row[:], 1.0)

        # full-sequence KV + context accumulators
        kT_sb = kv.tile([P, S], bf)                  # [head_dim', s]
        v_sb = kv.tile([P, S], bf)                   # [s%P, (s//P)*HD + hd]
        ctx_sb = kv.tile([P, NH_LOC * S], bf)        # [hd, h*S + sq]

        def stream_x(G):
            """DMA the 8 xT chunks of group G; returns per-k slices.

            Buffers persist for the whole group (bufs=10 > 8 chunks) so the
            head-major projection sweeps can re-read every k-slice."""
            slices = []
            for c2 in range(KT * SG // XCH):
                xw = xp.tile([P, XCH], bf, tag="xt", bufs=10, name="xt")
                blk = G * KT * SG + c2 * XCH
                if G == 0 and c2 == 0:
                    # split the very first chunk so matmul k=0 starts on a
                    # small lead transfer (subtile deps fire per region)
                    nc.sync.dma_start(xw[:, 0:SG], xt_d[:, blk:blk + SG])
                    nc.sync.dma_start(xw[:, SG:XCH],
                                      xt_d[:, blk + SG:blk + XCH])
                else:
                    nc.sync.dma_start(xw[:], xt_d[:, blk:blk + XCH])
                for j in range(XCH // SG):
                    slices.append(xw[:, j * SG:(j + 1) * SG])
            return slices

        def rope(src, dr, di, G):
            gsl = slice(G * SG, (G + 1) * SG)
            cos = cs_sb[0:64, gsl]
            sin = cs_sb[64:128, gsl]
            ta = rp.tile([64, SG], f32, tag="ropeA", bufs=2)
            tb = rp.tile([64, SG], f32, tag="ropeB", bufs=2)
            nc.vector.tensor_mul(ta[:], src[0:64, :], cos)
            nc.vector.tensor_mul(tb[:], src[64:128, :], sin)
            nc.vector.tensor_sub(dr, ta[:], tb[:])
            tc2 = rp.tile([64, SG], f32, tag="ropeA", bufs=2)
            td = rp.tile([64, SG], f32, tag="ropeB", bufs=2)
            nc.vector.tensor_mul(tc2[:], src[0:64, :], sin)
            nc.vector.tensor_mul(td[:], src[64:128, :], cos)
            nc.vector.tensor_add(di, tc2[:], td[:])

        # ---------------- per-group passes, interleaved just-in-time -------
        # PSUM budget (8 banks): "pair" [P,2*SG] x2 (4 banks, all projection
        # and score pairs), "bank" [P,SG] x3 (cacc + broadcast + pending
        # cacc), "sacc" [P,SG] x1 (4 heads' softmax sums packed at partition
        # offsets 0/32/64/96 via tile_position).
        def pass_kv(G, xs):
            """K/V projections + RoPE-k + V transpose for s-slice G."""
            if pending[0] is not None:
                finalize(pending[0])
                pending[0] = None
            pkv = ps.tile([P, 2 * SG], f32, tag="pair", bufs=2, name="pkv")
            pk = pkv[:, 0:SG]
            pv = pkv[:, SG:2 * SG]
            # alternate banks every matmul: same-bank back-to-back
            # accumulation serializes the PE at the isolated-MM rate
            for k in range(KT):
                st_k, sp_k = (k == 0), (k == KT - 1)
                nc.tensor.matmul(pk, wk_sb[:, k * HD:(k + 1) * HD], xs[k],
                                 start=st_k, stop=sp_k)
                nc.tensor.matmul(pv, wv_sb[:, k * HD:(k + 1) * HD], xs[k],
                                 start=st_k, stop=sp_k)
            gsl = slice(G * SG, (G + 1) * SG)
            rope(pk, kT_sb[0:64, gsl], kT_sb[64:128, gsl], G)
            vt = sp.tile([P, SG], bf, tag="vtmp", bufs=2)
            nc.scalar.copy(vt[:], pv)
            for j in range(SG // P):
                nc.scalar.dma_start_transpose(
                    v_sb[:, (G * 4 + j) * HD:(G * 4 + j + 1) * HD],
                    vt[:, j * P:(j + 1) * P])

        qts = {}

        def pass_q(G, xs):
            # head pairs: banks alternate every matmul (pipelining), yet each
            # pair's RoPE starts on the DVE half a group early, so the DVE
            # queue is clear when the attention pass needs it (mask-muls,
            # finalize). xs is shared with pass_kv(G): the chunk buffers
            # hold the whole group, so xT is read from DRAM exactly once.
            if pending[0] is not None:
                finalize(pending[0])
                pending[0] = None
            for lp in range(NH_LOC // 2):
                pqp = ps.tile([P, 2 * SG], f32, tag="pair", bufs=2, name="pqp")
                pq = [pqp[:, 0:SG], pqp[:, SG:2 * SG]]
                for k in range(KT):
                    for i in range(2):
                        l = 2 * lp + i
                        nc.tensor.matmul(
                            pq[i],
                            wq_sb[:, (k * NH_LOC + l) * HD:(k * NH_LOC + l + 1) * HD],
                            xs[k], start=(k == 0), stop=(k == KT - 1))
                for i in range(2):
                    dst = qp.tile([P, SG], bf, tag="qT", bufs=10, name="qT")
                    qts[(G, 2 * lp + i)] = dst
                    rope(pq[i], dst[0:64, :], dst[64:128, :], G)

        # pending per-head softmax finalization (deferred one head so the
        # PE never waits on the DVE reciprocal chain)
        def finalize(fin):
            cacc, sacc, h, G0 = fin
            t1 = sp.tile([1, SG], f32, tag="lns", bufs=2)
            nc.vector.reciprocal_approx_fast(t1[:], sacc)
            inv = sp.tile([1, SG], bf, tag="inv", bufs=2)
            nc.vector.tensor_copy(inv[:], t1[:])
            bcp = ps.tile([P, SG], f32, tag="sacc", bufs=2, name="bcp")
            nc.tensor.matmul(bcp[:], ones_row[:], inv[:], start=True, stop=True)
            bcs = sp.tile([P, SG], f32, tag="bcs", bufs=2)
            nc.vector.tensor_copy(bcs[:], bcp[:])
            nc.vector.tensor_mul(
                ctx_sb[:, h * S + G0 * SG:h * S + (G0 + 1) * SG],
                cacc[:], bcs[:])

        pending = [None]

        def pass_b(G):
            lst = sk_lists[G]
            # unmasked tiles first: masked tiles' extra DVE hop rides the
            # lookahead slack at the tail of the head instead of stalling
            # its first PV
            lst = [e for e in lst if e[1] is None] + \
                  [e for e in lst if e[1] is not None]
            n_sk = len(lst)
            npair = (n_sk + 1) // 2

            for h in range(NH_LOC):
                cacc = ps.tile([P, SG], f32, tag="bank", bufs=2, name="cacc")
                sacct = ps.tile([1, SG], f32, tag="sacc", bufs=2, name="sacct")
                sacc = sacct[:]

                def emit_pair(p):
                    # two score tiles into adjacent PSUM banks, ONE exp over
                    # both: the ACT engine's per-op overhead halves, keeping
                    # its per-tile rate under the PE's 3-matmul m-step
                    w = 2 if 2 * p + 1 < n_sk else 1
                    stp = ps.tile([P, 2 * SG], f32, tag="pair", bufs=2,
                                  name="stp")
                    for j in range(w):
                        m, pat = lst[2 * p + j]
                        nc.tensor.matmul(stp[:, j * SG:(j + 1) * SG],
                                         kT_sb[:, m * P:(m + 1) * P],
                                         qts[(G, h)][:], start=True, stop=True)
                    ex = ep.tile([P, 2 * SG], bf, tag="ex", bufs=3)
                    nc.scalar.activation(ex[:, 0:w * SG], stp[:, 0:w * SG], Exp)
                    for j in range(w):
                        m, pat = lst[2 * p + j]
                        if pat is not None:
                            nc.vector.tensor_mul(
                                ex[:, j * SG:(j + 1) * SG],
                                ex[:, j * SG:(j + 1) * SG],
                                mb_sb[:, pat * SG:(pat + 1) * SG])
                    return ex

                exq = [emit_pair(p) for p in range(min(2, npair))]
                # the deferred finalize of the previous head goes AFTER this
                # head's first score pairs: its broadcast matmul's DVE
                # reciprocal latency hides behind the score work
                if pending[0] is not None:
                    finalize(pending[0])
                    pending[0] = None
                for p in range(npair):
                    ex = exq[p]
                    w = 2 if 2 * p + 1 < n_sk else 1
                    # batch by stationary width: every transition between the
                    # 1-wide ones weights and 128-wide weights costs ~93ns of
                    # lost LDWEIGHTS prefetch, so do both cacc then both sacc
                    for j in range(w):
                        idx = 2 * p + j
                        m, pat = lst[idx]
                        nc.tensor.matmul(cacc[:], v_sb[:, m * HD:(m + 1) * HD],
                                         ex[:, j * SG:(j + 1) * SG],
                                         start=(idx == 0), stop=(idx == n_sk - 1))
                    for j in range(w):
                        idx = 2 * p + j
                        nc.tensor.matmul(sacc, ones_col[:],
                                         ex[:, j * SG:(j + 1) * SG],
                                         start=(idx == 0), stop=(idx == n_sk - 1))
                    # scores for pair p+2 AFTER the PVs of pair p: the PVs
                    # already synchronized on exp(p) via ex, so reusing
                    # pair p's PSUM slot here costs no extra wait
                    if p + 2 < npair:
                        exq.append(emit_pair(p + 2))
                pending[0] = (cacc, sacc, h, G)

        # schedule: kv(G)+q(G) share one xT stream (6 matmuls per k-tile --
        # never DMA-starved); attention lags its q-group by one so RoPE and
        # PSUM drains overlap PE work; next group's stream is dispatched
        # before the attention segment so it prefetches during it.
        load_wq(0)
        xs0 = stream_x(0)
        pass_kv(0, xs0)
        load_wq(1)
        pass_q(0, xs0)
        xs1 = stream_x(1)
        pass_kv(1, xs1)
        load_mb()
        pass_q(1, xs1)
        xs2 = stream_x(2)
        pass_b(0)
        load_wo()
        pass_kv(2, xs2)
        pass_q(2, xs2)
        xs3 = stream_x(3)
        pass_b(1)
        pass_kv(3, xs3)
        pass_q(3, xs3)
        pass_b(2)
        pass_b(3)
        finalize(pending[0])
        pending[0] = None

        # ---------------- pass C: out = ctx @ wo (partial) ----------------
        for m in range(NSK):
            for half in range(2):
                orow = cp.tile([P, D // 2], bf, tag="orow", bufs=2)
                # two output tiles per pair so accumulation alternates banks
                for n2 in range(0, D // SG // 2, 2):
                    n = half * (D // SG // 2) + n2
                    pop = ps.tile([P, 2 * SG], f32, tag="pair", bufs=2,
                                  name="pop")
                    po = [pop[:, 0:SG], pop[:, SG:2 * SG]]
                    for kk in range(NH_LOC):
                        for i in range(2):
                            nc.tensor.matmul(
                                po[i],
                                ctx_sb[:, kk * S + m * P:kk * S + (m + 1) * P],
                                wo_sb[:, ((n + i) * NH_LOC + kk) * SG:
                                         ((n + i) * NH_LOC + kk + 1) * SG],
                                start=(kk == 0), stop=(kk == NH_LOC - 1))
                    for i in range(2):
                        dst = orow[:, (n2 + i) * SG:(n2 + i + 1) * SG]
                        if i:
                            nc.scalar.copy(dst, po[i])
                        else:
                            nc.vector.tensor_copy(dst, po[i])
                nc.sync.dma_start(
                    out_d[m * P:(m + 1) * P, half * (D // 2):(half + 1) * (D // 2)],
                    orow[:])

    nc.compile()
    return nc


def _host_prep(x, wq, wk, wv, wo, freqs_cos, freqs_sin):
    """Build per-core input maps (all layouts pre-tiled for contiguous DMA)."""
    from concourse import mybir
    BF = np.dtype(mybir.dt.np(mybir.dt.bfloat16))

    x = np.ascontiguousarray(np.asarray(x, dtype=np.float32).reshape(S, D))
    wq = np.asarray(wq, dtype=np.float32)
    wk = np.asarray(wk, dtype=np.float32)
    wv = np.asarray(wv, dtype=np.float32)
    wo = np.asarray(wo, dtype=np.float32)

    perm = np.concatenate([np.arange(0, HD, 2), np.arange(1, HD, 2)])
    scale = 1.0 / math.sqrt(HD)
    wq_p = (wq.reshape(D, N_HEADS, HD)[:, :, perm] * scale).astype(np.float32)
    wk_p = wk.reshape(D, N_KV, HD)[:, :, perm]

    # xT blocks: xtb[p, G, k, c] = x[G*SG + c, k*P + p]
    xtb = np.ascontiguousarray(
        x.T.reshape(KT, P, NG, SG).transpose(1, 2, 0, 3)).astype(BF)
    xtb = np.ascontiguousarray(xtb.reshape(P, NG * KT * SG))
    cs = np.ascontiguousarray(
        np.concatenate([np.asarray(freqs_cos, np.float32).T,
                        np.asarray(freqs_sin, np.float32).T], axis=0))

    in_maps = []
    for c in range(N_CORES):
        wq_c = wq_p[:, 4 * c:4 * c + 4, :].reshape(D, NH_LOC * HD)
        wq_l = np.ascontiguousarray(
            wq_c.reshape(KT, P, NH_LOC * HD).transpose(1, 0, 2)
            .reshape(P, KT * NH_LOC * HD)).astype(BF)
        wk_c = wk_p[:, c, :]
        wk_l = np.ascontiguousarray(
            wk_c.reshape(KT, P, HD).transpose(1, 0, 2).reshape(P, KT * HD)).astype(BF)
        wv_c = wv.reshape(D, N_KV, HD)[:, c, :]
        wv_l = np.ascontiguousarray(
            wv_c.reshape(KT, P, HD).transpose(1, 0, 2).reshape(P, KT * HD)).astype(BF)
        wo_c = wo[4 * c * HD:(4 * c + 4) * HD, :]       # [512, D]
        # [P, n, kk, 512]: per dim-group n, the 4 head-chunk tiles adjacent
        wo_l = np.ascontiguousarray(
            wo_c.reshape(NH_LOC, P, D // SG, SG).transpose(1, 2, 0, 3)
            .reshape(P, (D // SG) * NH_LOC * SG)).astype(BF)
        in_maps.append({"xt": xtb, "wq": wq_l, "wk": wk_l,
                        "wv": wv_l, "wo": wo_l, "cs": cs})
    return in_maps


def _run(x, wq, wk, wv, wo, freqs_cos, freqs_sin, mask, start_pos, trace=False):
    assert int(start_pos) == 0
    from concourse import mybir
    BF = np.dtype(mybir.dt.np(mybir.dt.bfloat16))
    sk_lists, patterns = _classify_mask(np.asarray(mask, dtype=np.float32))
    n_pat = len(patterns)
    fp = (tuple(tuple(lst) for lst in sk_lists), n_pat)

    if fp not in _CACHE:
        _CACHE[fp] = _build_program(sk_lists, n_pat)
    nc = _CACHE[fp]

    in_maps = _host_prep(x, wq, wk, wv, wo, freqs_cos, freqs_sin)
    if n_pat:
        mb = np.ascontiguousarray(np.stack(patterns)).astype(BF)
        for m in in_maps:
            m["mb"] = mb

    from concourse.bass_utils import run_bass_kernel_spmd
    res = run_bass_kernel_spmd(nc, in_maps, list(range(N_CORES)), trace=trace)
    out = np.zeros((S, D), dtype=np.float32)
    for c in range(N_CORES):
        out += res.results[c]["out"].astype(np.float32)
    return out.reshape(1, S, D), res


def kernel(x, wq, wk, wv, wo, freqs_cos, freqs_sin, mask, start_pos):
    out, _ = _run(x, wq, wk, wv, wo, freqs_cos, freqs_sin, mask, start_pos)
    return out
